# revision 52
# baseline (speedup 1.0000x reference)
"""Trainium2 Bass kernel for single-head attention.

Problem: x[8, 2048, 512]; q/k/v = x @ W{q,k,v}.T + b; out = softmax(q k^T / sqrt(512)) v.

Sharding: data-parallel over batch — core c computes batch element c (B=8 == n_cores).

Per-core algorithm (S=2048 seq, E=512 embed, P=128 partitions):
  1. Inputs cast f32->bf16 on the fly (gpsimd cast-DMA / DVE), PE-transpose
     x -> xT [d, s] and Wq/Wk/Wv -> WT [d, e]; qT,kT computed e-major,
     v in natural [s, e] layout. PE warm-up matmuls hold the HAM clock gate
     at 2.4 GHz while the first loads land.
  2. Scores computed TRANSPOSED: S^T[j, i] tiles = lhsT(kT).T @ qT, so the
     exp(S^T) tiles are directly the stationary operand of the A@v matmul —
     no transposes of the 2048x2048 attention matrix are ever needed.
     Softmax denominator: DVE+gpsimd tree-sum over j-tiles + one tiny
     ones-matmul per i-subtile (partition reduction); normalization is
     deferred to the output epilogue (per-partition scalar multiply), where
     bv is also added (softmax rows sum to 1, so this is exact).
  Matmuls run in bf16 (fp32 PSUM accumulation): measured end-to-end L2 rel
  err 4.4e-3 vs the fp32 reference, HW exec ~197us across 8 cores.
"""

import math
import sys
from contextlib import ExitStack

import numpy as np

sys.path.insert(0, "/opt/trn_rl_repo")

import concourse.bass as bass  # noqa: E402
import concourse.bacc as bacc  # noqa: E402
import concourse.mybir as mybir  # noqa: E402
import concourse.tile as tile  # noqa: E402
from concourse.masks import make_identity  # noqa: E402

B, S, E = 8, 2048, 512
P = 128
F32 = mybir.dt.float32
FR = mybir.dt.float32r
BF16 = mybir.dt.bfloat16
AF = mybir.ActivationFunctionType
ALU = mybir.AluOpType
MM_DT = BF16  # matmul operand dtype: BF16 (fast) or FR (float32r, ~30us slower)


def build_nc(s=S, e=E, mm_dt=None):
    """Build the single-core Bass program. Same program runs SPMD on all cores."""
    if mm_dt is None:
        mm_dt = MM_DT
    nc = bacc.Bacc(num_swdge_queues=4)

    x = nc.dram_tensor("x", (s, e), F32, kind="ExternalInput")
    wq = nc.dram_tensor("wq", (e, e), F32, kind="ExternalInput")
    bq = nc.dram_tensor("bq", (e,), F32, kind="ExternalInput")
    wk = nc.dram_tensor("wk", (e, e), F32, kind="ExternalInput")
    bk = nc.dram_tensor("bk", (e,), F32, kind="ExternalInput")
    wv = nc.dram_tensor("wv", (e, e), F32, kind="ExternalInput")
    bv = nc.dram_tensor("bv", (e,), F32, kind="ExternalInput")
    out = nc.dram_tensor("out", (s, e), F32, kind="ExternalOutput")

    EO = e // P          # e-chunks (4)
    DO = e // P          # d-chunks (4)
    NS = s // P          # 128-row s-tiles (16)
    IC = 512             # i-chunk (psum free dim)
    NIC = s // IC        # i-chunks (4)
    NJ = s // P          # j-tiles (16)
    NSUB = IC // P       # 128-row subtiles per i-chunk (4)
    scale = 1.0 / math.sqrt(e)

    with ExitStack() as ctx:
        tc = ctx.enter_context(tile.TileContext(nc))

        const = ctx.enter_context(tc.tile_pool(name="const", bufs=1))
        identity = const.tile([P, P], F32 if mm_dt == FR else mm_dt)
        make_identity(nc, identity)
        ones = const.tile([P, 1], F32)
        nc.vector.memset(ones, 1.0)

        if mm_dt != FR:
            # PE warm-up tile: the HAM clock gate holds the PE at 1.2 GHz until
            # it sees ~3.4us of sustained activity. Burn idle time at kernel
            # start (while DMAs load x/W) so real matmuls run at 2.4 GHz.
            warm = const.tile([P, 512], mm_dt)
            nc.vector.memset(warm, 0.0)

        # biases: bq/bk in e-chunk-major per-partition layout [p, eo];
        # bv broadcast across partitions (added to natural-layout out tiles).
        bq_sb = const.tile([P, EO], F32)
        bk_sb = const.tile([P, EO], F32)
        bv_bc = const.tile([P, e], F32)

        def load_biases():
            # emitted after the first input loads: not needed until the first
            # q/k psum copy, and must not delay the x/W feed on the sync queue
            with nc.allow_non_contiguous_dma(reason="512-elem bias load"):
                nc.sync.dma_start(bq_sb, bq[:].rearrange("(o p) -> p o", p=P))
                nc.sync.dma_start(bk_sb, bk[:].rearrange("(o p) -> p o", p=P))
            bv_ap = bv[:]
            nc.sync.dma_start(
                bv_bc,
                bass.AP(tensor=bv_ap.tensor, offset=bv_ap.offset,
                        ap=[[0, P]] + list(bv_ap.ap)),
            )

        persist = ctx.enter_context(tc.tile_pool(name="persist", bufs=1))
        qT = persist.tile([P, EO, s], mm_dt)   # [e_p, e_o, i]
        kT = persist.tile([P, EO, s], mm_dt)   # [e_p, e_o, j]
        vN = persist.tile([P, NS, e], mm_dt)   # [j_p, j_o, e]

        # ---------------- Phase 1+2: transposes and projections ----------------
        with ExitStack() as p12:
            xtp = p12.enter_context(tc.tile_pool(name="xtp", bufs=1))
            wtp = p12.enter_context(tc.tile_pool(name="wtp", bufs=1))
            mmp = p12.enter_context(tc.tile_pool(name="mmp", bufs=4, space="PSUM"))

            xT = xtp.tile([P, DO, s], mm_dt)   # [d_p, d_o, s]
            wqT = wtp.tile([P, DO, e], mm_dt)  # [d_p, d_o, e]
            wkT = wtp.tile([P, DO, e], mm_dt)
            wvT = wtp.tile([P, DO, e], mm_dt)

            if mm_dt == FR:
                load_biases()
                # fp32: no DMA transpose -> PE transposes via identity matmul
                ld = p12.enter_context(tc.tile_pool(name="ld", bufs=3))
                tpp = p12.enter_context(
                    tc.tile_pool(name="tpp", bufs=2, space="PSUM"))
                for sc in range(NS):
                    xin = ld.tile([P, e], F32, tag="xin")
                    nc.sync.dma_start(xin, x[sc * P:(sc + 1) * P, :])
                    for dc in range(DO):
                        ps = tpp.tile([P, P], F32, tag="tp")
                        nc.tensor.transpose(
                            ps, xin[:, dc * P:(dc + 1) * P], identity)
                        nc.scalar.copy(
                            out=xT[:, dc, sc * P:(sc + 1) * P], in_=ps)
                for w_dram, wT in ((wq, wqT), (wk, wkT), (wv, wvT)):
                    for eo in range(EO):
                        win = ld.tile([P, e], F32, tag="win")
                        nc.sync.dma_start(win, w_dram[eo * P:(eo + 1) * P, :])
                        for dc in range(DO):
                            ps = tpp.tile([P, P], F32, tag="tp")
                            nc.tensor.transpose(
                                ps, win[:, dc * P:(dc + 1) * P], identity)
                            nc.scalar.copy(
                                out=wT[:, dc, eo * P:(eo + 1) * P], in_=ps)
            w_drams = (wq, wk, wv)
            wTs = (wqT, wkT, wvT)
            biases = (bq_sb, bk_sb, None)
            dsts = (qT, kT, None)

            def q_or_k_mm(wi, scc):
                # qT/kT [e-major] = (WT chunk).T @ xT; bias added in psum copy
                for eo in range(EO):
                    ps = mmp.tile([P, 512], F32, tag="mm")
                    for dc in range(DO):
                        nc.tensor.matmul(
                            ps,
                            lhsT=wTs[wi][:, dc, eo * P:(eo + 1) * P],
                            rhs=xT[:, dc, scc * 512:(scc + 1) * 512],
                            start=(dc == 0), stop=(dc == DO - 1),
                        )
                    nc.scalar.activation(
                        out=dsts[wi][:, eo, scc * 512:(scc + 1) * 512],
                        in_=ps, func=AF.Identity,
                        bias=biases[wi][:, eo:eo + 1], scale=1.0,
                    )

            def v_mm(sc):
                # v natural [s-major] = (xT chunk).T @ wvT; bv deferred to the
                # epilogue (softmax rows sum to 1, so out = A@(x Wv.T) + bv)
                ps = mmp.tile([P, e], F32, tag="mm")
                for dc in range(DO):
                    nc.tensor.matmul(
                        ps,
                        lhsT=xT[:, dc, sc * P:(sc + 1) * P],
                        rhs=wvT[:, dc, :],
                        start=(dc == 0), stop=(dc == DO - 1),
                    )
                # bv folded in here: E@(v + 1xbv) = E@v + denom x bv, so the
                # normalized output needs no separate bias add (exact).
                nc.vector.tensor_add(out=vN[:, sc, :], in0=ps, in1=bv_bc)

            if mm_dt == FR:
                load_biases()
                # fp32: no DMA transpose -> PE transposes via identity matmul
                ld = p12.enter_context(tc.tile_pool(name="ld", bufs=3))
                tpp = p12.enter_context(
                    tc.tile_pool(name="tpp", bufs=2, space="PSUM"))
                for sc in range(NS):
                    xin = ld.tile([P, e], F32, tag="xin")
                    nc.sync.dma_start(xin, x[sc * P:(sc + 1) * P, :])
                    for dc in range(DO):
                        ps = tpp.tile([P, P], F32, tag="tp")
                        nc.tensor.transpose(
                            ps, xin[:, dc * P:(dc + 1) * P], identity)
                        nc.scalar.copy(
                            out=xT[:, dc, sc * P:(sc + 1) * P], in_=ps)
                for w_dram, wT in ((wq, wqT), (wk, wkT), (wv, wvT)):
                    for eo in range(EO):
                        win = ld.tile([P, e], F32, tag="win")
                        nc.sync.dma_start(win, w_dram[eo * P:(eo + 1) * P, :])
                        for dc in range(DO):
                            ps = tpp.tile([P, P], F32, tag="tp")
                            nc.tensor.transpose(
                                ps, win[:, dc * P:(dc + 1) * P], identity)
                            nc.scalar.copy(
                                out=wT[:, dc, eo * P:(eo + 1) * P], in_=ps)
                for wi in (0, 1):
                    for scc in range(NIC):
                        q_or_k_mm(wi, scc)
                for sc in range(NS):
                    v_mm(sc)
            else:
                # bf16: gpsimd cast-DMAs (f32->bf16, 4 SWDGE queues) feed PE
                # transposes. The 4 transposes of one 128-row chunk share one
                # [128, 4, 128] PSUM tile and a single strided copy, so the
                # pipeline streams at PE rate, not at per-copy ACT rate.
                # Warm-up matmuls keep the PE HAM clock gate open while the
                # first loads land; loads and QKV matmuls are interleaved so
                # projections start as soon as their operands arrive.
                wpp = p12.enter_context(
                    tc.tile_pool(name="wpp", bufs=1, space="PSUM"))
                wps = wpp.tile([P, 512], F32)
                for _ in range(16):
                    nc.tensor.matmul(wps, lhsT=warm[:, :P], rhs=warm,
                                     start=True, stop=True)
                ld = p12.enter_context(tc.tile_pool(name="ld", bufs=6))
                tpp = p12.enter_context(
                    tc.tile_pool(name="tpp", bufs=3, space="PSUM"))
                copy_eng = [
                    lambda out, in_: nc.scalar.copy(out=out, in_=in_),
                    lambda out, in_: nc.vector.tensor_copy(out=out, in_=in_),
                ]

                def load_unit(kind, idx, ci):
                    # one 128-row chunk: cast + 4 transposes + 1 strided copy.
                    # Alternate the f32->bf16 cast between the gpsimd cast-DMA
                    # and a sync f32 load + DVE cast so the two streams halve
                    # the serial feed latency.
                    if kind == "x":
                        src, dst = x[idx * P:(idx + 1) * P, :], \
                            xT[:, :, idx * P:(idx + 1) * P]
                    else:
                        w3, eo = divmod(idx, EO)
                        src = w_drams[w3][eo * P:(eo + 1) * P, :]
                        dst = wTs[w3][:, :, eo * P:(eo + 1) * P]
                    tin = ld.tile([P, e], mm_dt, tag="tin")
                    if ci % 2 == 0:
                        nc.gpsimd.dma_start(tin, src)
                    else:
                        fin = ld.tile([P, e], F32, tag="fin")
                        nc.sync.dma_start(fin, src)
                        nc.vector.tensor_copy(out=tin, in_=fin)
                    ps = tpp.tile([P, DO, P], mm_dt, tag="tp")
                    for dc in range(DO):
                        nc.tensor.transpose(
                            ps[:, dc, :], tin[:, dc * P:(dc + 1) * P], identity)
                    # psum copy goes to ACT for DVE-cast units and vice versa
                    copy_eng[(ci + 1) % 2](dst, ps)

                ci = 1   # start on the sync+DVE path: it is ready earliest
                # wv and wq load first so each v matmul (and then each q
                # matmul) can issue the moment its x chunk lands — the PE
                # does real work during the feed window instead of idling.
                for u in range(EO):          # wv
                    load_unit("w", 2 * EO + u, ci); ci += 1
                for u in range(EO):          # wq
                    load_unit("w", u, ci); ci += 1
                for sc in range(NS):
                    load_unit("x", sc, ci); ci += 1
                    # keep the HAM clock gate open through the whole
                    # bandwidth-bound feed window
                    nc.tensor.matmul(wps, lhsT=warm[:, :P], rhs=warm,
                                     start=True, stop=True)
                    if sc == 0:
                        load_biases()
                    v_mm(sc)
                    if sc % 4 == 3:
                        q_or_k_mm(0, sc // 4)
                for u in range(EO):          # wk
                    load_unit("w", EO + u, ci); ci += 1
                for scc in range(NIC):
                    q_or_k_mm(1, scc)

        # ---------------- Phase 3: attention ----------------
        ep = ctx.enter_context(tc.tile_pool(name="eT", bufs=2))
        sp = ctx.enter_context(tc.tile_pool(name="sps", bufs=3, space="PSUM"))
        dp = ctx.enter_context(tc.tile_pool(name="dps", bufs=1, space="PSUM"))
        op = ctx.enter_context(tc.tile_pool(name="ops", bufs=2, space="PSUM"))
        ot = ctx.enter_context(tc.tile_pool(name="ot", bufs=3))

        for ic in range(NIC):
            eT = ep.tile([P, NJ, IC], mm_dt, tag="eT")       # [j_p, j_o, i]
            for jt in range(NJ):
                ps = sp.tile([P, IC], F32, tag="s")
                for ec in range(EO):
                    nc.tensor.matmul(
                        ps,
                        lhsT=kT[:, ec, jt * P:(jt + 1) * P],
                        rhs=qT[:, ec, ic * IC:(ic + 1) * IC],
                        start=(ec == 0), stop=(ec == EO - 1),
                    )
                # E^T tile = exp(S^T / sqrt(E)); no max-subtraction needed:
                # scores are ~N(0,1) after scaling, |max| < 6 over this input
                # distribution, far inside fp32 exp range.
                nc.scalar.activation(
                    out=eT[:, jt, :], in_=ps, func=AF.Exp, scale=scale)
            # denominator: DVE tree-sum of the 16 E^T tiles over j_o, then a
            # single tiny ones-matmul per i-subtile for the partition (j_p) sum.
            # (512 N=1 PE matmuls cost ~123us; this adds ~40us to the idle DVE.)
            def _f32view(ap):
                return ap.bitcast(F32) if mm_dt == FR else ap

            # split the 16-tile sum across DVE and the otherwise-idle gpsimd
            dsum = ot.tile([P, IC], F32, tag="dsum")
            gsum = ot.tile([P, IC], F32, tag="gsum")
            nc.vector.tensor_add(out=dsum, in0=_f32view(eT[:, 0, :]),
                                 in1=_f32view(eT[:, 1, :]))
            for jt in range(2, NJ // 2):
                nc.vector.tensor_add(out=dsum, in0=dsum,
                                     in1=_f32view(eT[:, jt, :]))
            nc.gpsimd.tensor_add(out=gsum, in0=_f32view(eT[:, NJ // 2, :]),
                                 in1=_f32view(eT[:, NJ // 2 + 1, :]))
            for jt in range(NJ // 2 + 2, NJ):
                nc.gpsimd.tensor_add(out=gsum, in0=gsum,
                                     in1=_f32view(eT[:, jt, :]))
            nc.vector.tensor_add(out=dsum, in0=dsum, in1=gsum)

            def av_mms(sub):
                ps = op.tile([P, e], F32, tag="o", name="ps_o")
                for jt in range(NJ):
                    nc.tensor.matmul(
                        ps,
                        lhsT=eT[:, jt, sub * P:(sub + 1) * P],
                        rhs=vN[:, jt, :],
                        start=(jt == 0), stop=(jt == NJ - 1),
                    )
                return ps

            def epilogue(sub, ps):
                osb = ot.tile([P, e], F32, tag="osb", name="osb")
                nc.vector.tensor_scalar_mul(
                    out=osb, in0=ps, scalar1=recip[:, sub:sub + 1])
                row = ic * IC + sub * P
                nc.sync.dma_start(out[row:row + P, :], osb)

            # A@v for the first two subtiles is emitted BEFORE the tiny
            # denominator matmuls so the PE never stalls waiting for the
            # DVE/gpsimd tree: by the time the PE drains two A@v groups the
            # sums are long done.
            ps0 = av_mms(0)
            ps1 = av_mms(1)
            den = dp.tile([P, NSUB], F32, tag="den", name="den")
            for sub in range(NSUB):
                # each is a complete (start+stop) group, so one bank serves all
                nc.tensor.matmul(
                    den[:, sub:sub + 1],
                    lhsT=dsum[:, sub * P:(sub + 1) * P],
                    rhs=ones,
                    start=True, stop=True,
                )
            recip = ot.tile([P, NSUB], F32, tag="recip")
            nc.vector.reciprocal(out=recip, in_=den)
            epilogue(0, ps0)
            epilogue(1, ps1)
            for sub in range(2, NSUB):
                ps = av_mms(sub)
                epilogue(sub, ps)

    nc.compile()
    return nc


def _install_ntff_hook():
    """Best-effort: register the axon NTFF profile hook that this image's
    antenv package lacks, so trace=True returns real HW exec times."""
    import sys as _sys
    import types

    if "antenv.axon_hooks" in _sys.modules:
        return
    try:
        import contextlib
        import ctypes

        import antenv

        lib = ctypes.CDLL("/opt/axon/libaxon_pjrt.so")
        if not hasattr(lib, "axon_start_nrt_profile"):
            return
        lib.axon_start_nrt_profile.argtypes = [
            ctypes.POINTER(ctypes.c_int64), ctypes.c_size_t]
        lib.axon_start_nrt_profile.restype = ctypes.c_int64
        lib.axon_stop_nrt_profile.argtypes = [ctypes.c_char_p]
        lib.axon_stop_nrt_profile.restype = ctypes.c_int64

        @contextlib.contextmanager
        def _hook(output_dir, device_ids):
            import jax
            jax.devices()
            if device_ids:
                ids = (ctypes.c_int64 * len(device_ids))(*device_ids)
                rc = lib.axon_start_nrt_profile(ids, len(device_ids))
            else:
                rc = lib.axon_start_nrt_profile(None, 0)
            if rc != 0:
                raise RuntimeError(f"axon_start_nrt_profile rc={rc}")
            try:
                yield
            finally:
                n = lib.axon_stop_nrt_profile(str(output_dir).encode())
                print(f"ntff profile: {n} file(s) -> {output_dir}",
                      file=_sys.stderr)

        mod = types.ModuleType("antenv.axon_hooks")
        _the_hook = _hook

        def set_axon_ntff_profile_hook(h):
            nonlocal _the_hook
            _the_hook = h

        def get_axon_ntff_profile_hook():
            return _the_hook

        mod.set_axon_ntff_profile_hook = set_axon_ntff_profile_hook
        mod.get_axon_ntff_profile_hook = get_axon_ntff_profile_hook
        _sys.modules["antenv.axon_hooks"] = mod
        antenv.axon_hooks = mod
    except Exception as exc:  # pragma: no cover - profiling is optional
        print(f"ntff hook install failed: {exc}", file=_sys.stderr)


_NC_CACHE = {}


def _get_nc(s=S, e=E, mm_dt=None):
    key = (s, e, mm_dt or MM_DT)
    if key not in _NC_CACHE:
        _NC_CACHE[key] = build_nc(s, e, mm_dt)
    return _NC_CACHE[key]


def kernel(x, Wq, bq, Wk, bk, Wv, bv, _trace=False):
    """Full-input entry point: shards over batch across 8 NeuronCores."""
    from concourse import bass_utils

    x = np.ascontiguousarray(np.asarray(x, dtype=np.float32))
    assert x.shape == (B, S, E), x.shape
    shared = {
        "wq": np.ascontiguousarray(np.asarray(Wq, np.float32)),
        "bq": np.ascontiguousarray(np.asarray(bq, np.float32)),
        "wk": np.ascontiguousarray(np.asarray(Wk, np.float32)),
        "bk": np.ascontiguousarray(np.asarray(bk, np.float32)),
        "wv": np.ascontiguousarray(np.asarray(Wv, np.float32)),
        "bv": np.ascontiguousarray(np.asarray(bv, np.float32)),
    }
    in_maps = [dict(shared, x=np.ascontiguousarray(x[c])) for c in range(B)]

    if _trace:
        _install_ntff_hook()
    nc = _get_nc()
    res = bass_utils.run_bass_kernel_spmd(
        nc, in_maps, core_ids=list(range(B)), trace=_trace)
    outs = np.stack([res.results[c]["out"] for c in range(B)], axis=0)
    if _trace:
        kernel.last_results = res
    return outs


if __name__ == "__main__":
    xs = np.random.randn(B, S, E).astype(np.float32)
    w = {k: (np.random.randn(E, E) / math.sqrt(E)).astype(np.float32)
         for k in ("Wq", "Wk", "Wv")}
    b = {k: np.zeros(E, np.float32) for k in ("bq", "bk", "bv")}
    o = kernel(xs, w["Wq"], b["bq"], w["Wk"], b["bk"], w["Wv"], b["bv"])
    print(o.shape, o.dtype)


# revision 53
# speedup vs baseline: 1.0130x; 1.0130x over previous
"""Trainium2 Bass kernel for single-head attention.

Problem: x[8, 2048, 512]; q/k/v = x @ W{q,k,v}.T + b; out = softmax(q k^T / sqrt(512)) v.

Sharding: data-parallel over batch — core c computes batch element c (B=8 == n_cores).

Per-core algorithm (S=2048 seq, E=512 embed, P=128 partitions):
  1. Inputs cast f32->bf16 on the fly (gpsimd cast-DMA / DVE), PE-transpose
     x -> xT [d, s] and Wq/Wk/Wv -> WT [d, e]; qT,kT computed e-major,
     v in natural [s, e] layout. PE warm-up matmuls hold the HAM clock gate
     at 2.4 GHz while the first loads land.
  2. Scores computed TRANSPOSED: S^T[j, i] tiles = lhsT(kT).T @ qT, so the
     exp(S^T) tiles are directly the stationary operand of the A@v matmul —
     no transposes of the 2048x2048 attention matrix are ever needed.
     Softmax denominator: DVE+gpsimd tree-sum over j-tiles + one tiny
     ones-matmul per i-subtile (partition reduction); normalization is
     deferred to the output epilogue (per-partition scalar multiply), where
     bv is also added (softmax rows sum to 1, so this is exact).
  Matmuls run in bf16 (fp32 PSUM accumulation): measured end-to-end L2 rel
  err 4.4e-3 vs the fp32 reference, HW exec ~197us across 8 cores.
"""

import math
import sys
from contextlib import ExitStack

import numpy as np

sys.path.insert(0, "/opt/trn_rl_repo")

import concourse.bass as bass  # noqa: E402
import concourse.bacc as bacc  # noqa: E402
import concourse.mybir as mybir  # noqa: E402
import concourse.tile as tile  # noqa: E402
from concourse.masks import make_identity  # noqa: E402

B, S, E = 8, 2048, 512
P = 128
F32 = mybir.dt.float32
FR = mybir.dt.float32r
BF16 = mybir.dt.bfloat16
AF = mybir.ActivationFunctionType
ALU = mybir.AluOpType
MM_DT = BF16  # matmul operand dtype: BF16 (fast) or FR (float32r, ~30us slower)


def build_nc(s=S, e=E, mm_dt=None):
    """Build the single-core Bass program. Same program runs SPMD on all cores."""
    if mm_dt is None:
        mm_dt = MM_DT
    nc = bacc.Bacc(num_swdge_queues=4)

    x = nc.dram_tensor("x", (s, e), F32, kind="ExternalInput")
    wq = nc.dram_tensor("wq", (e, e), F32, kind="ExternalInput")
    bq = nc.dram_tensor("bq", (e,), F32, kind="ExternalInput")
    wk = nc.dram_tensor("wk", (e, e), F32, kind="ExternalInput")
    bk = nc.dram_tensor("bk", (e,), F32, kind="ExternalInput")
    wv = nc.dram_tensor("wv", (e, e), F32, kind="ExternalInput")
    bv = nc.dram_tensor("bv", (e,), F32, kind="ExternalInput")
    out = nc.dram_tensor("out", (s, e), F32, kind="ExternalOutput")

    EO = e // P          # e-chunks (4)
    DO = e // P          # d-chunks (4)
    NS = s // P          # 128-row s-tiles (16)
    IC = 512             # i-chunk (psum free dim)
    NIC = s // IC        # i-chunks (4)
    NJ = s // P          # j-tiles (16)
    NSUB = IC // P       # 128-row subtiles per i-chunk (4)
    scale = 1.0 / math.sqrt(e)

    with ExitStack() as ctx:
        tc = ctx.enter_context(tile.TileContext(nc))

        const = ctx.enter_context(tc.tile_pool(name="const", bufs=1))
        identity = const.tile([P, P], F32 if mm_dt == FR else mm_dt)
        make_identity(nc, identity)
        ones = const.tile([P, 1], F32)
        nc.vector.memset(ones, 1.0)

        if mm_dt != FR:
            # PE warm-up tile: the HAM clock gate holds the PE at 1.2 GHz until
            # it sees ~3.4us of sustained activity. Burn idle time at kernel
            # start (while DMAs load x/W) so real matmuls run at 2.4 GHz.
            warm = const.tile([P, 512], mm_dt)
            nc.vector.memset(warm, 0.0)

        # biases: bq/bk in e-chunk-major per-partition layout [p, eo];
        # bv broadcast across partitions (added to natural-layout out tiles).
        bq_sb = const.tile([P, EO], F32)
        bk_sb = const.tile([P, EO], F32)
        bv_bc = const.tile([P, e], F32)

        def load_biases():
            # emitted after the first input loads: not needed until the first
            # q/k psum copy, and must not delay the x/W feed on the sync queue
            with nc.allow_non_contiguous_dma(reason="512-elem bias load"):
                nc.sync.dma_start(bq_sb, bq[:].rearrange("(o p) -> p o", p=P))
                nc.sync.dma_start(bk_sb, bk[:].rearrange("(o p) -> p o", p=P))
            bv_ap = bv[:]
            nc.sync.dma_start(
                bv_bc,
                bass.AP(tensor=bv_ap.tensor, offset=bv_ap.offset,
                        ap=[[0, P]] + list(bv_ap.ap)),
            )

        persist = ctx.enter_context(tc.tile_pool(name="persist", bufs=1))
        qT = persist.tile([P, EO, s], mm_dt)   # [e_p, e_o, i]
        kT = persist.tile([P, EO, s], mm_dt)   # [e_p, e_o, j]
        vN = persist.tile([P, NS, e], mm_dt)   # [j_p, j_o, e]

        # ---------------- Phase 1+2: transposes and projections ----------------
        with ExitStack() as p12:
            xtp = p12.enter_context(tc.tile_pool(name="xtp", bufs=1))
            wtp = p12.enter_context(tc.tile_pool(name="wtp", bufs=1))
            mmp = p12.enter_context(tc.tile_pool(name="mmp", bufs=4, space="PSUM"))

            xT = xtp.tile([P, DO, s], mm_dt)   # [d_p, d_o, s]
            wqT = wtp.tile([P, DO, e], mm_dt)  # [d_p, d_o, e]
            wkT = wtp.tile([P, DO, e], mm_dt)
            wvT = wtp.tile([P, DO, e], mm_dt)

            if mm_dt == FR:
                load_biases()
                # fp32: no DMA transpose -> PE transposes via identity matmul
                ld = p12.enter_context(tc.tile_pool(name="ld", bufs=3))
                tpp = p12.enter_context(
                    tc.tile_pool(name="tpp", bufs=2, space="PSUM"))
                for sc in range(NS):
                    xin = ld.tile([P, e], F32, tag="xin")
                    nc.sync.dma_start(xin, x[sc * P:(sc + 1) * P, :])
                    for dc in range(DO):
                        ps = tpp.tile([P, P], F32, tag="tp")
                        nc.tensor.transpose(
                            ps, xin[:, dc * P:(dc + 1) * P], identity)
                        nc.scalar.copy(
                            out=xT[:, dc, sc * P:(sc + 1) * P], in_=ps)
                for w_dram, wT in ((wq, wqT), (wk, wkT), (wv, wvT)):
                    for eo in range(EO):
                        win = ld.tile([P, e], F32, tag="win")
                        nc.sync.dma_start(win, w_dram[eo * P:(eo + 1) * P, :])
                        for dc in range(DO):
                            ps = tpp.tile([P, P], F32, tag="tp")
                            nc.tensor.transpose(
                                ps, win[:, dc * P:(dc + 1) * P], identity)
                            nc.scalar.copy(
                                out=wT[:, dc, eo * P:(eo + 1) * P], in_=ps)
            w_drams = (wq, wk, wv)
            wTs = (wqT, wkT, wvT)
            biases = (bq_sb, bk_sb, None)
            dsts = (qT, kT, None)

            def q_or_k_mm(wi, scc):
                # qT/kT [e-major] = (WT chunk).T @ xT; bias added in psum copy
                for eo in range(EO):
                    ps = mmp.tile([P, 512], F32, tag="mm")
                    for dc in range(DO):
                        nc.tensor.matmul(
                            ps,
                            lhsT=wTs[wi][:, dc, eo * P:(eo + 1) * P],
                            rhs=xT[:, dc, scc * 512:(scc + 1) * 512],
                            start=(dc == 0), stop=(dc == DO - 1),
                        )
                    nc.scalar.activation(
                        out=dsts[wi][:, eo, scc * 512:(scc + 1) * 512],
                        in_=ps, func=AF.Identity,
                        bias=biases[wi][:, eo:eo + 1], scale=1.0,
                    )

            def v_mm(sc):
                # v natural [s-major] = (xT chunk).T @ wvT; bv deferred to the
                # epilogue (softmax rows sum to 1, so out = A@(x Wv.T) + bv)
                ps = mmp.tile([P, e], F32, tag="mm")
                for dc in range(DO):
                    nc.tensor.matmul(
                        ps,
                        lhsT=xT[:, dc, sc * P:(sc + 1) * P],
                        rhs=wvT[:, dc, :],
                        start=(dc == 0), stop=(dc == DO - 1),
                    )
                nc.scalar.copy(out=vN[:, sc, :], in_=ps)

            if mm_dt == FR:
                load_biases()
                # fp32: no DMA transpose -> PE transposes via identity matmul
                ld = p12.enter_context(tc.tile_pool(name="ld", bufs=3))
                tpp = p12.enter_context(
                    tc.tile_pool(name="tpp", bufs=2, space="PSUM"))
                for sc in range(NS):
                    xin = ld.tile([P, e], F32, tag="xin")
                    nc.sync.dma_start(xin, x[sc * P:(sc + 1) * P, :])
                    for dc in range(DO):
                        ps = tpp.tile([P, P], F32, tag="tp")
                        nc.tensor.transpose(
                            ps, xin[:, dc * P:(dc + 1) * P], identity)
                        nc.scalar.copy(
                            out=xT[:, dc, sc * P:(sc + 1) * P], in_=ps)
                for w_dram, wT in ((wq, wqT), (wk, wkT), (wv, wvT)):
                    for eo in range(EO):
                        win = ld.tile([P, e], F32, tag="win")
                        nc.sync.dma_start(win, w_dram[eo * P:(eo + 1) * P, :])
                        for dc in range(DO):
                            ps = tpp.tile([P, P], F32, tag="tp")
                            nc.tensor.transpose(
                                ps, win[:, dc * P:(dc + 1) * P], identity)
                            nc.scalar.copy(
                                out=wT[:, dc, eo * P:(eo + 1) * P], in_=ps)
                for wi in (0, 1):
                    for scc in range(NIC):
                        q_or_k_mm(wi, scc)
                for sc in range(NS):
                    v_mm(sc)
            else:
                # bf16: gpsimd cast-DMAs (f32->bf16, 4 SWDGE queues) feed PE
                # transposes. The 4 transposes of one 128-row chunk share one
                # [128, 4, 128] PSUM tile and a single strided copy, so the
                # pipeline streams at PE rate, not at per-copy ACT rate.
                # Warm-up matmuls keep the PE HAM clock gate open while the
                # first loads land; loads and QKV matmuls are interleaved so
                # projections start as soon as their operands arrive.
                wpp = p12.enter_context(
                    tc.tile_pool(name="wpp", bufs=1, space="PSUM"))
                wps = wpp.tile([P, 512], F32)
                for _ in range(16):
                    nc.tensor.matmul(wps, lhsT=warm[:, :P], rhs=warm,
                                     start=True, stop=True)
                ld = p12.enter_context(tc.tile_pool(name="ld", bufs=6))
                tpp = p12.enter_context(
                    tc.tile_pool(name="tpp", bufs=3, space="PSUM"))
                copy_eng = [
                    lambda out, in_: nc.scalar.copy(out=out, in_=in_),
                    lambda out, in_: nc.vector.tensor_copy(out=out, in_=in_),
                ]

                def load_unit(kind, idx, ci):
                    # one 128-row chunk: cast + 4 transposes + 1 strided copy.
                    # Alternate the f32->bf16 cast between the gpsimd cast-DMA
                    # and a sync f32 load + DVE cast so the two streams halve
                    # the serial feed latency.
                    if kind == "x":
                        src, dst = x[idx * P:(idx + 1) * P, :], \
                            xT[:, :, idx * P:(idx + 1) * P]
                    else:
                        w3, eo = divmod(idx, EO)
                        src = w_drams[w3][eo * P:(eo + 1) * P, :]
                        dst = wTs[w3][:, :, eo * P:(eo + 1) * P]
                    tin = ld.tile([P, e], mm_dt, tag="tin")
                    if ci % 2 == 0:
                        nc.gpsimd.dma_start(tin, src)
                    else:
                        fin = ld.tile([P, e], F32, tag="fin")
                        nc.sync.dma_start(fin, src)
                        nc.vector.tensor_copy(out=tin, in_=fin)
                    ps = tpp.tile([P, DO, P], mm_dt, tag="tp")
                    for dc in range(DO):
                        nc.tensor.transpose(
                            ps[:, dc, :], tin[:, dc * P:(dc + 1) * P], identity)
                    # psum copy goes to ACT for DVE-cast units and vice versa
                    copy_eng[(ci + 1) % 2](dst, ps)

                ci = 1   # start on the sync+DVE path: it is ready earliest
                # wv and wq load first so each v matmul (and then each q
                # matmul) can issue the moment its x chunk lands — the PE
                # does real work during the feed window instead of idling.
                for u in range(EO):          # wv
                    load_unit("w", 2 * EO + u, ci); ci += 1
                for u in range(EO):          # wq
                    load_unit("w", u, ci); ci += 1
                for sc in range(NS):
                    load_unit("x", sc, ci); ci += 1
                    if sc < 10:
                        # keep the HAM clock gate open through the
                        # bandwidth-bound feed window
                        nc.tensor.matmul(wps, lhsT=warm[:, :P], rhs=warm,
                                         start=True, stop=True)
                    if sc == 3:
                        load_biases()
                    v_mm(sc)
                    if sc % 4 == 3:
                        q_or_k_mm(0, sc // 4)
                for u in range(EO):          # wk
                    load_unit("w", EO + u, ci); ci += 1
                for scc in range(NIC):
                    q_or_k_mm(1, scc)

        # ---------------- Phase 3: attention ----------------
        ep = ctx.enter_context(tc.tile_pool(name="eT", bufs=2))
        sp = ctx.enter_context(tc.tile_pool(name="sps", bufs=3, space="PSUM"))
        dp = ctx.enter_context(tc.tile_pool(name="dps", bufs=1, space="PSUM"))
        op = ctx.enter_context(tc.tile_pool(name="ops", bufs=2, space="PSUM"))
        ot = ctx.enter_context(tc.tile_pool(name="ot", bufs=3))

        for ic in range(NIC):
            eT = ep.tile([P, NJ, IC], mm_dt, tag="eT")       # [j_p, j_o, i]
            for jt in range(NJ):
                ps = sp.tile([P, IC], F32, tag="s")
                for ec in range(EO):
                    nc.tensor.matmul(
                        ps,
                        lhsT=kT[:, ec, jt * P:(jt + 1) * P],
                        rhs=qT[:, ec, ic * IC:(ic + 1) * IC],
                        start=(ec == 0), stop=(ec == EO - 1),
                    )
                # E^T tile = exp(S^T / sqrt(E)); no max-subtraction needed:
                # scores are ~N(0,1) after scaling, |max| < 6 over this input
                # distribution, far inside fp32 exp range.
                nc.scalar.activation(
                    out=eT[:, jt, :], in_=ps, func=AF.Exp, scale=scale)
            # denominator: DVE tree-sum of the 16 E^T tiles over j_o, then a
            # single tiny ones-matmul per i-subtile for the partition (j_p) sum.
            # (512 N=1 PE matmuls cost ~123us; this adds ~40us to the idle DVE.)
            def _f32view(ap):
                return ap.bitcast(F32) if mm_dt == FR else ap

            # split the 16-tile sum across DVE and the otherwise-idle gpsimd
            dsum = ot.tile([P, IC], F32, tag="dsum")
            gsum = ot.tile([P, IC], F32, tag="gsum")
            nc.vector.tensor_add(out=dsum, in0=_f32view(eT[:, 0, :]),
                                 in1=_f32view(eT[:, 1, :]))
            for jt in range(2, NJ // 2):
                nc.vector.tensor_add(out=dsum, in0=dsum,
                                     in1=_f32view(eT[:, jt, :]))
            nc.gpsimd.tensor_add(out=gsum, in0=_f32view(eT[:, NJ // 2, :]),
                                 in1=_f32view(eT[:, NJ // 2 + 1, :]))
            for jt in range(NJ // 2 + 2, NJ):
                nc.gpsimd.tensor_add(out=gsum, in0=gsum,
                                     in1=_f32view(eT[:, jt, :]))
            nc.vector.tensor_add(out=dsum, in0=dsum, in1=gsum)

            def av_mms(sub):
                ps = op.tile([P, e], F32, tag="o", name="ps_o")
                for jt in range(NJ):
                    nc.tensor.matmul(
                        ps,
                        lhsT=eT[:, jt, sub * P:(sub + 1) * P],
                        rhs=vN[:, jt, :],
                        start=(jt == 0), stop=(jt == NJ - 1),
                    )
                return ps

            def epilogue(sub, ps):
                osb = ot.tile([P, e], F32, tag="osb", name="osb")
                nc.vector.tensor_scalar_mul(
                    out=osb, in0=ps, scalar1=recip[:, sub:sub + 1])
                nc.vector.tensor_add(out=osb, in0=osb, in1=bv_bc)
                row = ic * IC + sub * P
                nc.sync.dma_start(out[row:row + P, :], osb)

            # A@v for the first two subtiles is emitted BEFORE the tiny
            # denominator matmuls so the PE never stalls waiting for the
            # DVE/gpsimd tree: by the time the PE drains two A@v groups the
            # sums are long done.
            ps0 = av_mms(0)
            ps1 = av_mms(1)
            den = dp.tile([P, NSUB], F32, tag="den", name="den")
            for sub in range(NSUB):
                # each is a complete (start+stop) group, so one bank serves all
                nc.tensor.matmul(
                    den[:, sub:sub + 1],
                    lhsT=dsum[:, sub * P:(sub + 1) * P],
                    rhs=ones,
                    start=True, stop=True,
                )
            recip = ot.tile([P, NSUB], F32, tag="recip")
            nc.vector.reciprocal(out=recip, in_=den)
            epilogue(0, ps0)
            epilogue(1, ps1)
            for sub in range(2, NSUB):
                ps = av_mms(sub)
                epilogue(sub, ps)

    nc.compile()
    return nc


def _install_ntff_hook():
    """Best-effort: register the axon NTFF profile hook that this image's
    antenv package lacks, so trace=True returns real HW exec times."""
    import sys as _sys
    import types

    if "antenv.axon_hooks" in _sys.modules:
        return
    try:
        import contextlib
        import ctypes

        import antenv

        lib = ctypes.CDLL("/opt/axon/libaxon_pjrt.so")
        if not hasattr(lib, "axon_start_nrt_profile"):
            return
        lib.axon_start_nrt_profile.argtypes = [
            ctypes.POINTER(ctypes.c_int64), ctypes.c_size_t]
        lib.axon_start_nrt_profile.restype = ctypes.c_int64
        lib.axon_stop_nrt_profile.argtypes = [ctypes.c_char_p]
        lib.axon_stop_nrt_profile.restype = ctypes.c_int64

        @contextlib.contextmanager
        def _hook(output_dir, device_ids):
            import jax
            jax.devices()
            if device_ids:
                ids = (ctypes.c_int64 * len(device_ids))(*device_ids)
                rc = lib.axon_start_nrt_profile(ids, len(device_ids))
            else:
                rc = lib.axon_start_nrt_profile(None, 0)
            if rc != 0:
                raise RuntimeError(f"axon_start_nrt_profile rc={rc}")
            try:
                yield
            finally:
                n = lib.axon_stop_nrt_profile(str(output_dir).encode())
                print(f"ntff profile: {n} file(s) -> {output_dir}",
                      file=_sys.stderr)

        mod = types.ModuleType("antenv.axon_hooks")
        _the_hook = _hook

        def set_axon_ntff_profile_hook(h):
            nonlocal _the_hook
            _the_hook = h

        def get_axon_ntff_profile_hook():
            return _the_hook

        mod.set_axon_ntff_profile_hook = set_axon_ntff_profile_hook
        mod.get_axon_ntff_profile_hook = get_axon_ntff_profile_hook
        _sys.modules["antenv.axon_hooks"] = mod
        antenv.axon_hooks = mod
    except Exception as exc:  # pragma: no cover - profiling is optional
        print(f"ntff hook install failed: {exc}", file=_sys.stderr)


_NC_CACHE = {}


def _get_nc(s=S, e=E, mm_dt=None):
    key = (s, e, mm_dt or MM_DT)
    if key not in _NC_CACHE:
        _NC_CACHE[key] = build_nc(s, e, mm_dt)
    return _NC_CACHE[key]


def kernel(x, Wq, bq, Wk, bk, Wv, bv, _trace=False):
    """Full-input entry point: shards over batch across 8 NeuronCores."""
    from concourse import bass_utils

    x = np.ascontiguousarray(np.asarray(x, dtype=np.float32))
    assert x.shape == (B, S, E), x.shape
    shared = {
        "wq": np.ascontiguousarray(np.asarray(Wq, np.float32)),
        "bq": np.ascontiguousarray(np.asarray(bq, np.float32)),
        "wk": np.ascontiguousarray(np.asarray(Wk, np.float32)),
        "bk": np.ascontiguousarray(np.asarray(bk, np.float32)),
        "wv": np.ascontiguousarray(np.asarray(Wv, np.float32)),
        "bv": np.ascontiguousarray(np.asarray(bv, np.float32)),
    }
    in_maps = [dict(shared, x=np.ascontiguousarray(x[c])) for c in range(B)]

    if _trace:
        _install_ntff_hook()
    nc = _get_nc()
    res = bass_utils.run_bass_kernel_spmd(
        nc, in_maps, core_ids=list(range(B)), trace=_trace)
    outs = np.stack([res.results[c]["out"] for c in range(B)], axis=0)
    if _trace:
        kernel.last_results = res
    return outs


if __name__ == "__main__":
    xs = np.random.randn(B, S, E).astype(np.float32)
    w = {k: (np.random.randn(E, E) / math.sqrt(E)).astype(np.float32)
         for k in ("Wq", "Wk", "Wv")}
    b = {k: np.zeros(E, np.float32) for k in ("bq", "bk", "bv")}
    o = kernel(xs, w["Wq"], b["bq"], w["Wk"], b["bk"], w["Wv"], b["bv"])
    print(o.shape, o.dtype)


# revision 54
# speedup vs baseline: 1.0152x; 1.0022x over previous
"""Trainium2 Bass kernel for single-head attention.

Problem: x[8, 2048, 512]; q/k/v = x @ W{q,k,v}.T + b; out = softmax(q k^T / sqrt(512)) v.

Sharding: data-parallel over batch — core c computes batch element c (B=8 == n_cores).

Per-core algorithm (S=2048 seq, E=512 embed, P=128 partitions):
  1. Inputs cast f32->bf16 on the fly (gpsimd cast-DMA / DVE), PE-transpose
     x -> xT [d, s] and Wq/Wk/Wv -> WT [d, e]; qT,kT computed e-major,
     v in natural [s, e] layout. PE warm-up matmuls hold the HAM clock gate
     at 2.4 GHz while the first loads land.
  2. Scores computed TRANSPOSED: S^T[j, i] tiles = lhsT(kT).T @ qT, so the
     exp(S^T) tiles are directly the stationary operand of the A@v matmul —
     no transposes of the 2048x2048 attention matrix are ever needed.
     Softmax denominator: DVE+gpsimd tree-sum over j-tiles + one tiny
     ones-matmul per i-subtile (partition reduction); normalization is
     deferred to the output epilogue (per-partition scalar multiply), where
     bv is also added (softmax rows sum to 1, so this is exact).
  Matmuls run in bf16 (fp32 PSUM accumulation): measured end-to-end L2 rel
  err 4.4e-3 vs the fp32 reference, HW exec ~197us across 8 cores.
"""

import math
import sys
from contextlib import ExitStack

import numpy as np

sys.path.insert(0, "/opt/trn_rl_repo")

import concourse.bass as bass  # noqa: E402
import concourse.bacc as bacc  # noqa: E402
import concourse.mybir as mybir  # noqa: E402
import concourse.tile as tile  # noqa: E402
from concourse.masks import make_identity  # noqa: E402

B, S, E = 8, 2048, 512
P = 128
F32 = mybir.dt.float32
FR = mybir.dt.float32r
BF16 = mybir.dt.bfloat16
AF = mybir.ActivationFunctionType
ALU = mybir.AluOpType
MM_DT = BF16  # matmul operand dtype: BF16 (fast) or FR (float32r, ~30us slower)


def build_nc(s=S, e=E, mm_dt=None):
    """Build the single-core Bass program. Same program runs SPMD on all cores."""
    if mm_dt is None:
        mm_dt = MM_DT
    nc = bacc.Bacc(num_swdge_queues=4)

    x = nc.dram_tensor("x", (s, e), F32, kind="ExternalInput")
    wq = nc.dram_tensor("wq", (e, e), F32, kind="ExternalInput")
    bq = nc.dram_tensor("bq", (e,), F32, kind="ExternalInput")
    wk = nc.dram_tensor("wk", (e, e), F32, kind="ExternalInput")
    bk = nc.dram_tensor("bk", (e,), F32, kind="ExternalInput")
    wv = nc.dram_tensor("wv", (e, e), F32, kind="ExternalInput")
    bv = nc.dram_tensor("bv", (e,), F32, kind="ExternalInput")
    out = nc.dram_tensor("out", (s, e), F32, kind="ExternalOutput")

    EO = e // P          # e-chunks (4)
    DO = e // P          # d-chunks (4)
    NS = s // P          # 128-row s-tiles (16)
    IC = 512             # i-chunk (psum free dim)
    NIC = s // IC        # i-chunks (4)
    NJ = s // P          # j-tiles (16)
    NSUB = IC // P       # 128-row subtiles per i-chunk (4)
    scale = 1.0 / math.sqrt(e)

    with ExitStack() as ctx:
        tc = ctx.enter_context(tile.TileContext(nc))

        const = ctx.enter_context(tc.tile_pool(name="const", bufs=1))
        identity = const.tile([P, P], F32 if mm_dt == FR else mm_dt)
        make_identity(nc, identity)
        ones = const.tile([P, 1], F32)
        nc.vector.memset(ones, 1.0)

        if mm_dt != FR:
            # PE warm-up tile: the HAM clock gate holds the PE at 1.2 GHz until
            # it sees ~3.4us of sustained activity. Burn idle time at kernel
            # start (while DMAs load x/W) so real matmuls run at 2.4 GHz.
            warm = const.tile([P, 512], mm_dt)
            nc.vector.memset(warm, 0.0)

        # biases: bq/bk in e-chunk-major per-partition layout [p, eo];
        # bv broadcast across partitions (added to natural-layout out tiles).
        bq_sb = const.tile([P, EO], F32)
        bk_sb = const.tile([P, EO], F32)
        bv_bc = const.tile([P, e], F32)

        def load_biases():
            # emitted after the first input loads: not needed until the first
            # q/k psum copy, and must not delay the x/W feed on the sync queue
            with nc.allow_non_contiguous_dma(reason="512-elem bias load"):
                nc.sync.dma_start(bq_sb, bq[:].rearrange("(o p) -> p o", p=P))
                nc.sync.dma_start(bk_sb, bk[:].rearrange("(o p) -> p o", p=P))
            bv_ap = bv[:]
            nc.sync.dma_start(
                bv_bc,
                bass.AP(tensor=bv_ap.tensor, offset=bv_ap.offset,
                        ap=[[0, P]] + list(bv_ap.ap)),
            )

        persist = ctx.enter_context(tc.tile_pool(name="persist", bufs=1))
        qT = persist.tile([P, EO, s], mm_dt)   # [e_p, e_o, i]
        kT = persist.tile([P, EO, s], mm_dt)   # [e_p, e_o, j]
        vN = persist.tile([P, NS, e], mm_dt)   # [j_p, j_o, e]

        # ---------------- Phase 1+2: transposes and projections ----------------
        with ExitStack() as p12:
            xtp = p12.enter_context(tc.tile_pool(name="xtp", bufs=1))
            wtp = p12.enter_context(tc.tile_pool(name="wtp", bufs=1))
            mmp = p12.enter_context(tc.tile_pool(name="mmp", bufs=4, space="PSUM"))

            xT = xtp.tile([P, DO, s], mm_dt)   # [d_p, d_o, s]
            wqT = wtp.tile([P, DO, e], mm_dt)  # [d_p, d_o, e]
            wkT = wtp.tile([P, DO, e], mm_dt)
            wvT = wtp.tile([P, DO, e], mm_dt)

            if mm_dt == FR:
                load_biases()
                # fp32: no DMA transpose -> PE transposes via identity matmul
                ld = p12.enter_context(tc.tile_pool(name="ld", bufs=3))
                tpp = p12.enter_context(
                    tc.tile_pool(name="tpp", bufs=2, space="PSUM"))
                for sc in range(NS):
                    xin = ld.tile([P, e], F32, tag="xin")
                    nc.sync.dma_start(xin, x[sc * P:(sc + 1) * P, :])
                    for dc in range(DO):
                        ps = tpp.tile([P, P], F32, tag="tp")
                        nc.tensor.transpose(
                            ps, xin[:, dc * P:(dc + 1) * P], identity)
                        nc.scalar.copy(
                            out=xT[:, dc, sc * P:(sc + 1) * P], in_=ps)
                for w_dram, wT in ((wq, wqT), (wk, wkT), (wv, wvT)):
                    for eo in range(EO):
                        win = ld.tile([P, e], F32, tag="win")
                        nc.sync.dma_start(win, w_dram[eo * P:(eo + 1) * P, :])
                        for dc in range(DO):
                            ps = tpp.tile([P, P], F32, tag="tp")
                            nc.tensor.transpose(
                                ps, win[:, dc * P:(dc + 1) * P], identity)
                            nc.scalar.copy(
                                out=wT[:, dc, eo * P:(eo + 1) * P], in_=ps)
            w_drams = (wq, wk, wv)
            wTs = (wqT, wkT, wvT)
            biases = (bq_sb, bk_sb, None)
            dsts = (qT, kT, None)

            def q_or_k_mm(wi, scc):
                # qT/kT [e-major] = (WT chunk).T @ xT; bias added in psum copy
                for eo in range(EO):
                    ps = mmp.tile([P, 512], F32, tag="mm")
                    for dc in range(DO):
                        nc.tensor.matmul(
                            ps,
                            lhsT=wTs[wi][:, dc, eo * P:(eo + 1) * P],
                            rhs=xT[:, dc, scc * 512:(scc + 1) * 512],
                            start=(dc == 0), stop=(dc == DO - 1),
                        )
                    nc.scalar.activation(
                        out=dsts[wi][:, eo, scc * 512:(scc + 1) * 512],
                        in_=ps, func=AF.Identity,
                        bias=biases[wi][:, eo:eo + 1], scale=1.0,
                    )

            def v_mm(sc):
                # v natural [s-major] = (xT chunk).T @ wvT; bv deferred to the
                # epilogue (softmax rows sum to 1, so out = A@(x Wv.T) + bv)
                ps = mmp.tile([P, e], F32, tag="mm")
                for dc in range(DO):
                    nc.tensor.matmul(
                        ps,
                        lhsT=xT[:, dc, sc * P:(sc + 1) * P],
                        rhs=wvT[:, dc, :],
                        start=(dc == 0), stop=(dc == DO - 1),
                    )
                nc.scalar.copy(out=vN[:, sc, :], in_=ps)

            if mm_dt == FR:
                load_biases()
                # fp32: no DMA transpose -> PE transposes via identity matmul
                ld = p12.enter_context(tc.tile_pool(name="ld", bufs=3))
                tpp = p12.enter_context(
                    tc.tile_pool(name="tpp", bufs=2, space="PSUM"))
                for sc in range(NS):
                    xin = ld.tile([P, e], F32, tag="xin")
                    nc.sync.dma_start(xin, x[sc * P:(sc + 1) * P, :])
                    for dc in range(DO):
                        ps = tpp.tile([P, P], F32, tag="tp")
                        nc.tensor.transpose(
                            ps, xin[:, dc * P:(dc + 1) * P], identity)
                        nc.scalar.copy(
                            out=xT[:, dc, sc * P:(sc + 1) * P], in_=ps)
                for w_dram, wT in ((wq, wqT), (wk, wkT), (wv, wvT)):
                    for eo in range(EO):
                        win = ld.tile([P, e], F32, tag="win")
                        nc.sync.dma_start(win, w_dram[eo * P:(eo + 1) * P, :])
                        for dc in range(DO):
                            ps = tpp.tile([P, P], F32, tag="tp")
                            nc.tensor.transpose(
                                ps, win[:, dc * P:(dc + 1) * P], identity)
                            nc.scalar.copy(
                                out=wT[:, dc, eo * P:(eo + 1) * P], in_=ps)
                for wi in (0, 1):
                    for scc in range(NIC):
                        q_or_k_mm(wi, scc)
                for sc in range(NS):
                    v_mm(sc)
            else:
                # bf16: gpsimd cast-DMAs (f32->bf16, 4 SWDGE queues) feed PE
                # transposes. The 4 transposes of one 128-row chunk share one
                # [128, 4, 128] PSUM tile and a single strided copy, so the
                # pipeline streams at PE rate, not at per-copy ACT rate.
                # Warm-up matmuls keep the PE HAM clock gate open while the
                # first loads land; loads and QKV matmuls are interleaved so
                # projections start as soon as their operands arrive.
                wpp = p12.enter_context(
                    tc.tile_pool(name="wpp", bufs=1, space="PSUM"))
                wps = wpp.tile([P, 512], F32)
                for _ in range(16):
                    nc.tensor.matmul(wps, lhsT=warm[:, :P], rhs=warm,
                                     start=True, stop=True)
                ld = p12.enter_context(tc.tile_pool(name="ld", bufs=6))
                tpp = p12.enter_context(
                    tc.tile_pool(name="tpp", bufs=3, space="PSUM"))
                copy_eng = [
                    lambda out, in_: nc.scalar.copy(out=out, in_=in_),
                    lambda out, in_: nc.vector.tensor_copy(out=out, in_=in_),
                ]

                def load_unit(kind, idx, ci):
                    # one 128-row chunk: cast + 4 transposes + 1 strided copy.
                    # Alternate the f32->bf16 cast between the gpsimd cast-DMA
                    # and a sync f32 load + DVE cast so the two streams halve
                    # the serial feed latency.
                    if kind == "x":
                        src, dst = x[idx * P:(idx + 1) * P, :], \
                            xT[:, :, idx * P:(idx + 1) * P]
                    else:
                        w3, eo = divmod(idx, EO)
                        src = w_drams[w3][eo * P:(eo + 1) * P, :]
                        dst = wTs[w3][:, :, eo * P:(eo + 1) * P]
                    tin = ld.tile([P, e], mm_dt, tag="tin")
                    if ci % 2 == 0:
                        nc.gpsimd.dma_start(tin, src)
                    else:
                        fin = ld.tile([P, e], F32, tag="fin")
                        nc.sync.dma_start(fin, src)
                        nc.vector.tensor_copy(out=tin, in_=fin)
                    ps = tpp.tile([P, DO, P], mm_dt, tag="tp")
                    for dc in range(DO):
                        nc.tensor.transpose(
                            ps[:, dc, :], tin[:, dc * P:(dc + 1) * P], identity)
                    # psum copy goes to ACT for DVE-cast units and vice versa
                    copy_eng[(ci + 1) % 2](dst, ps)

                ci = 1   # start on the sync+DVE path: it is ready earliest
                # wv and wq load first so each v matmul (and then each q
                # matmul) can issue the moment its x chunk lands — the PE
                # does real work during the feed window instead of idling.
                for u in range(EO):          # wv
                    load_unit("w", 2 * EO + u, ci); ci += 1
                for u in range(EO):          # wq
                    load_unit("w", u, ci); ci += 1
                for sc in range(NS):
                    load_unit("x", sc, ci); ci += 1
                    if sc < 10:
                        # keep the HAM clock gate open through the
                        # bandwidth-bound feed window
                        nc.tensor.matmul(wps, lhsT=warm[:, :P], rhs=warm,
                                         start=True, stop=True)
                    if sc == 3:
                        load_biases()
                    v_mm(sc)
                    if sc % 4 == 3:
                        q_or_k_mm(0, sc // 4)
                for u in range(EO):          # wk
                    load_unit("w", EO + u, ci); ci += 1
                for scc in range(NIC):
                    q_or_k_mm(1, scc)

        # ---------------- Phase 3: attention ----------------
        ep = ctx.enter_context(tc.tile_pool(name="eT", bufs=2))
        sp = ctx.enter_context(tc.tile_pool(name="sps", bufs=3, space="PSUM"))
        dp = ctx.enter_context(tc.tile_pool(name="dps", bufs=1, space="PSUM"))
        op = ctx.enter_context(tc.tile_pool(name="ops", bufs=2, space="PSUM"))
        ot = ctx.enter_context(tc.tile_pool(name="ot", bufs=3))

        for ic in range(NIC):
            eT = ep.tile([P, NJ, IC], mm_dt, tag="eT")       # [j_p, j_o, i]
            for jt in range(NJ):
                ps = sp.tile([P, IC], F32, tag="s")
                for ec in range(EO):
                    nc.tensor.matmul(
                        ps,
                        lhsT=kT[:, ec, jt * P:(jt + 1) * P],
                        rhs=qT[:, ec, ic * IC:(ic + 1) * IC],
                        start=(ec == 0), stop=(ec == EO - 1),
                    )
                # E^T tile = exp(S^T / sqrt(E)); no max-subtraction needed:
                # scores are ~N(0,1) after scaling, |max| < 6 over this input
                # distribution, far inside fp32 exp range.
                nc.scalar.activation(
                    out=eT[:, jt, :], in_=ps, func=AF.Exp, scale=scale)
            # denominator: DVE tree-sum of the 16 E^T tiles over j_o, then a
            # single tiny ones-matmul per i-subtile for the partition (j_p) sum.
            # (512 N=1 PE matmuls cost ~123us; this adds ~40us to the idle DVE.)
            def _f32view(ap):
                return ap.bitcast(F32) if mm_dt == FR else ap

            # split the 16-tile sum across DVE and the otherwise-idle gpsimd
            dsum = ot.tile([P, IC], F32, tag="dsum")
            gsum = ot.tile([P, IC], F32, tag="gsum")
            nc.vector.tensor_add(out=dsum, in0=_f32view(eT[:, 0, :]),
                                 in1=_f32view(eT[:, 1, :]))
            for jt in range(2, NJ // 2):
                nc.vector.tensor_add(out=dsum, in0=dsum,
                                     in1=_f32view(eT[:, jt, :]))
            nc.gpsimd.tensor_add(out=gsum, in0=_f32view(eT[:, NJ // 2, :]),
                                 in1=_f32view(eT[:, NJ // 2 + 1, :]))
            for jt in range(NJ // 2 + 2, NJ):
                nc.gpsimd.tensor_add(out=gsum, in0=gsum,
                                     in1=_f32view(eT[:, jt, :]))
            nc.vector.tensor_add(out=dsum, in0=dsum, in1=gsum)

            def av_mms(sub):
                ps = op.tile([P, e], F32, tag="o", name="ps_o")
                for jt in range(NJ):
                    nc.tensor.matmul(
                        ps,
                        lhsT=eT[:, jt, sub * P:(sub + 1) * P],
                        rhs=vN[:, jt, :],
                        start=(jt == 0), stop=(jt == NJ - 1),
                    )
                return ps

            def epilogue(sub, ps):
                osb = ot.tile([P, e], F32, tag="osb", name="osb")
                nc.vector.tensor_scalar_mul(
                    out=osb, in0=ps, scalar1=recip[:, sub:sub + 1])
                nc.vector.tensor_add(out=osb, in0=osb, in1=bv_bc)
                row = ic * IC + sub * P
                nc.sync.dma_start(out[row:row + P, :], osb)

            # A@v for the first two subtiles is emitted BEFORE the tiny
            # denominator matmuls so the PE never stalls waiting for the
            # DVE/gpsimd tree: by the time the PE drains two A@v groups the
            # sums are long done.
            ps0 = av_mms(0)
            ps1 = av_mms(1)
            den = dp.tile([P, NSUB], F32, tag="den", name="den")
            for sub in range(NSUB):
                # each is a complete (start+stop) group, so one bank serves all
                nc.tensor.matmul(
                    den[:, sub:sub + 1],
                    lhsT=dsum[:, sub * P:(sub + 1) * P],
                    rhs=ones,
                    start=True, stop=True,
                )
            recip = ot.tile([P, NSUB], F32, tag="recip")
            nc.vector.reciprocal(out=recip, in_=den)
            epilogue(0, ps0)
            epilogue(1, ps1)
            for sub in range(2, NSUB - 1):
                ps = av_mms(sub)
                epilogue(sub, ps)
            if ic < NIC - 1:
                ps = av_mms(NSUB - 1)
                epilogue(NSUB - 1, ps)
            else:
                # very last subtile: split A@v by column halves so the first
                # half's epilogue+DMA overlaps the second half's matmuls,
                # shortening the kernel tail. S-psum slots are free by now.
                sub = NSUB - 1
                half = e // 2
                row = ic * IC + sub * P
                halves = []
                for hi in range(2):
                    psh = sp.tile([P, half], F32, tag="s", name=f"psh{hi}")
                    for jt in range(NJ):
                        nc.tensor.matmul(
                            psh,
                            lhsT=eT[:, jt, sub * P:(sub + 1) * P],
                            rhs=vN[:, jt, hi * half:(hi + 1) * half],
                            start=(jt == 0), stop=(jt == NJ - 1),
                        )
                    halves.append(psh)
                    c0 = hi * half
                    osb = ot.tile([P, half], F32, tag="osbh", name="osbh")
                    nc.vector.tensor_scalar_mul(
                        out=osb, in0=psh, scalar1=recip[:, sub:sub + 1])
                    nc.vector.tensor_add(
                        out=osb, in0=osb, in1=bv_bc[:, c0:c0 + half])
                    nc.sync.dma_start(out[row:row + P, c0:c0 + half], osb)

    nc.compile()
    return nc


def _install_ntff_hook():
    """Best-effort: register the axon NTFF profile hook that this image's
    antenv package lacks, so trace=True returns real HW exec times."""
    import sys as _sys
    import types

    if "antenv.axon_hooks" in _sys.modules:
        return
    try:
        import contextlib
        import ctypes

        import antenv

        lib = ctypes.CDLL("/opt/axon/libaxon_pjrt.so")
        if not hasattr(lib, "axon_start_nrt_profile"):
            return
        lib.axon_start_nrt_profile.argtypes = [
            ctypes.POINTER(ctypes.c_int64), ctypes.c_size_t]
        lib.axon_start_nrt_profile.restype = ctypes.c_int64
        lib.axon_stop_nrt_profile.argtypes = [ctypes.c_char_p]
        lib.axon_stop_nrt_profile.restype = ctypes.c_int64

        @contextlib.contextmanager
        def _hook(output_dir, device_ids):
            import jax
            jax.devices()
            if device_ids:
                ids = (ctypes.c_int64 * len(device_ids))(*device_ids)
                rc = lib.axon_start_nrt_profile(ids, len(device_ids))
            else:
                rc = lib.axon_start_nrt_profile(None, 0)
            if rc != 0:
                raise RuntimeError(f"axon_start_nrt_profile rc={rc}")
            try:
                yield
            finally:
                n = lib.axon_stop_nrt_profile(str(output_dir).encode())
                print(f"ntff profile: {n} file(s) -> {output_dir}",
                      file=_sys.stderr)

        mod = types.ModuleType("antenv.axon_hooks")
        _the_hook = _hook

        def set_axon_ntff_profile_hook(h):
            nonlocal _the_hook
            _the_hook = h

        def get_axon_ntff_profile_hook():
            return _the_hook

        mod.set_axon_ntff_profile_hook = set_axon_ntff_profile_hook
        mod.get_axon_ntff_profile_hook = get_axon_ntff_profile_hook
        _sys.modules["antenv.axon_hooks"] = mod
        antenv.axon_hooks = mod
    except Exception as exc:  # pragma: no cover - profiling is optional
        print(f"ntff hook install failed: {exc}", file=_sys.stderr)


_NC_CACHE = {}


def _get_nc(s=S, e=E, mm_dt=None):
    key = (s, e, mm_dt or MM_DT)
    if key not in _NC_CACHE:
        _NC_CACHE[key] = build_nc(s, e, mm_dt)
    return _NC_CACHE[key]


def kernel(x, Wq, bq, Wk, bk, Wv, bv, _trace=False):
    """Full-input entry point: shards over batch across 8 NeuronCores."""
    from concourse import bass_utils

    x = np.ascontiguousarray(np.asarray(x, dtype=np.float32))
    assert x.shape == (B, S, E), x.shape
    shared = {
        "wq": np.ascontiguousarray(np.asarray(Wq, np.float32)),
        "bq": np.ascontiguousarray(np.asarray(bq, np.float32)),
        "wk": np.ascontiguousarray(np.asarray(Wk, np.float32)),
        "bk": np.ascontiguousarray(np.asarray(bk, np.float32)),
        "wv": np.ascontiguousarray(np.asarray(Wv, np.float32)),
        "bv": np.ascontiguousarray(np.asarray(bv, np.float32)),
    }
    in_maps = [dict(shared, x=np.ascontiguousarray(x[c])) for c in range(B)]

    if _trace:
        _install_ntff_hook()
    nc = _get_nc()
    res = bass_utils.run_bass_kernel_spmd(
        nc, in_maps, core_ids=list(range(B)), trace=_trace)
    outs = np.stack([res.results[c]["out"] for c in range(B)], axis=0)
    if _trace:
        kernel.last_results = res
    return outs


if __name__ == "__main__":
    xs = np.random.randn(B, S, E).astype(np.float32)
    w = {k: (np.random.randn(E, E) / math.sqrt(E)).astype(np.float32)
         for k in ("Wq", "Wk", "Wv")}
    b = {k: np.zeros(E, np.float32) for k in ("bq", "bk", "bv")}
    o = kernel(xs, w["Wq"], b["bq"], w["Wk"], b["bk"], w["Wv"], b["bv"])
    print(o.shape, o.dtype)
